# revision 15
# baseline (speedup 1.0000x reference)
"""BitLinear-1.58 forward on 8 trn2 NeuronCores.

out = x @ qw.T + bias, qw = clip(round(w / (eps + mean|w|)), -1, 1).

Strategy (column-parallel, 1024 out-cols per core):
  - Ternary weights are exact in both bf16 and fp8e4 (values -1/0/1), so the
    only quantization error is on x. The 2048-wide contraction is split:
      * C_BF bf16 chunks (128 cols each): x in bf16, standard matmul,
        1 col/PE-cycle.
      * C_DR fp8 DoubleRow chunks (256 cols each): x in e4m3, 2 cols/PE-cycle
        (perf_mode=DoubleRow packs 2 fp8 rows per PE cell).
    Both chunk types accumulate into the same PSUM group. With C_DR=4 /
    C_BF=8 the kernel does 12 x 512-cycle-ish matmul passes per output tile
    instead of 16, a ~1.28x PE-time reduction, at rel_err ~1.7e-2 (vs the
    2e-2 gate; fp8 columns contribute ~2.65e-2 * sqrt(1024/2048)).
  - Per core: weights stay SBUF-resident (3MB); x streams through SBUF in
    64 token-tiles of 128 tokens; output [128, 1024] f32 written per tile.
"""

import numpy as np
import ml_dtypes

B, S, IN, OUT = 4, 2048, 2048, 8192
N_CORES = 8
TOK = B * S
NS = OUT // N_CORES          # out cols per core
SCALE_EPS = 1e-05

C_DR = 4                     # fp8 DoubleRow chunks (256 in-cols each)
C_BF = 16 - 2 * C_DR         # bf16 chunks (128 in-cols each)
N_DR = C_DR * 256            # fp8-covered in-cols (tail of the 2048)
N_BF = IN - N_DR             # bf16-covered in-cols (head)
TT = TOK // 128              # 64 token tiles
NH = NS // 512               # psum halves (2)

F8 = ml_dtypes.float8_e4m3   # TRN fp8e4 (max 240) == ml_dtypes.float8_e4m3
BF16 = ml_dtypes.bfloat16

_CACHED_NC = None


def _build_nc():
    import concourse.mybir as mybir
    import concourse.tile as tile
    from concourse import bacc

    DR = mybir.MatmulPerfMode.DoubleRow
    nc = bacc.Bacc(None, target_bir_lowering=False)

    xbf = nc.dram_tensor("xbf", [128, TT, C_BF, 128], mybir.dt.bfloat16,
                         kind="ExternalInput")
    wbf = nc.dram_tensor("wbf", [128, C_BF, NS], mybir.dt.bfloat16,
                         kind="ExternalInput")
    if C_DR:
        xdr = nc.dram_tensor("xdr", [128, TT, 2 * C_DR, 128],
                             mybir.dt.float8e4, kind="ExternalInput")
        wdr = nc.dram_tensor("wdr", [128, 2 * C_DR, NS], mybir.dt.float8e4,
                             kind="ExternalInput")
    out = nc.dram_tensor("out", [TOK, NS], mybir.dt.float32,
                         kind="ExternalOutput")

    with tile.TileContext(nc) as tc:
        with (
            tc.tile_pool(name="wres", bufs=1) as wres,
            tc.tile_pool(name="xb_pool", bufs=3) as xb_pool,
            tc.tile_pool(name="xd_pool", bufs=3) as xd_pool,
            tc.tile_pool(name="ostage", bufs=3) as ostage,
            tc.tile_pool(name="psum", bufs=3, space="PSUM") as psum,
            tc.tile_pool(name="warm", bufs=1) as warm_pool,
            tc.tile_pool(name="warm_psum", bufs=1, space="PSUM") as warm_psum,
        ):
            # resident weights; chunk-granular DMAs so compute can start as
            # soon as the first chunk lands
            wdr_t = None
            if C_DR:
                wdr_t = wres.tile([128, 2 * C_DR, NS], mybir.dt.float8e4)
            wbf_t = wres.tile([128, C_BF, NS], mybir.dt.bfloat16)
            # interleave chunk DMAs in consumption order so tile 0 never
            # waits on a weight chunk
            for c in range(max(C_DR, C_BF)):
                if c < C_DR:
                    nc.sync.dma_start(wdr_t[:, 2 * c:2 * c + 2, :],
                                      wdr[:, 2 * c:2 * c + 2, :])
                if c < C_BF:
                    nc.sync.dma_start(wbf_t[:, c:c + 1, :], wbf[:, c:c + 1, :])

            # PE warm-up: dummy matmuls with no data deps keep the PE busy
            # while the first tiles DMA in, releasing the HAM clock gate
            # (2.4 GHz) before the real matmul stream starts.
            wl = warm_pool.tile([128, 512], mybir.dt.bfloat16)
            wp = warm_psum.tile([128, 512], mybir.dt.float32)
            nc.vector.memset(wl[:], 0.0)
            n_warm = 8
            for i in range(n_warm):
                nc.tensor.matmul(
                    wp[:], wl[:, :128], wl[:], start=(i == 0),
                    stop=(i == n_warm - 1)
                )

            for t in range(TT):
                if C_DR:
                    xd = xd_pool.tile([128, 2 * C_DR, 128], mybir.dt.float8e4)
                    nc.scalar.dma_start(xd[:], xdr[:, t, :, :])
                xb = xb_pool.tile([128, C_BF, 128], mybir.dt.bfloat16)
                nc.scalar.dma_start(xb[:], xbf[:, t, :, :])
                ps = [psum.tile([128, 512], mybir.dt.float32, name=f"ps{h}")
                      for h in range(NH)]

                n_chunks = C_DR + C_BF
                ci = 0
                for c in range(C_DR):
                    lhsT = xd[:, 2 * c:2 * c + 2, :]
                    for h in range(NH):
                        nc.tensor.matmul(
                            ps[h][:],
                            lhsT,
                            wdr_t[:, 2 * c:2 * c + 2, h * 512:(h + 1) * 512],
                            start=(ci == 0), stop=(ci == n_chunks - 1),
                            perf_mode=DR,
                        )
                    ci += 1
                for c in range(C_BF):
                    lhsT = xb[:, c:c + 1, :]
                    for h in range(NH):
                        nc.tensor.matmul(
                            ps[h][:],
                            lhsT,
                            wbf_t[:, c:c + 1, h * 512:(h + 1) * 512],
                            start=(ci == 0), stop=(ci == n_chunks - 1),
                        )
                    ci += 1

                stage = ostage.tile([128, NS], mybir.dt.float32)
                for h in range(NH):
                    nc.any.tensor_copy(stage[:, h * 512:(h + 1) * 512],
                                       ps[h][:])
                nc.scalar.dma_start(out[t * 128:(t + 1) * 128, :], stage[:])

    nc.compile()
    return nc


def _get_nc():
    global _CACHED_NC
    if _CACHED_NC is None:
        _CACHED_NC = _build_nc()
    return _CACHED_NC


def _quantize_weight(weight: np.ndarray) -> np.ndarray:
    """Ternarize exactly as the reference does (same jax ops, same backend)."""
    import jax.numpy as jnp

    w = jnp.asarray(weight)
    scale = SCALE_EPS + jnp.mean(jnp.abs(w))
    quant = jnp.clip(jnp.round(w / scale), -1.0, 1.0)
    return np.asarray(quant, dtype=np.float32)


def _prepare_in_maps(x: np.ndarray, weight: np.ndarray):
    qw = _quantize_weight(weight)          # [OUT, IN] ternary fp32

    x2 = x.reshape(TT, 128, IN)            # [t, m, in]

    # bf16 chunks: in-cols [0, N_BF); layout [p, t, c, m]
    xb = x2[:, :, :N_BF].astype(BF16)      # [t, m, N_BF]
    xb = xb.reshape(TT, 128, C_BF, 128).transpose(3, 0, 2, 1)
    xb = np.ascontiguousarray(xb)

    # fp8 chunks: in-cols [N_BF, IN); col = N_BF + c*256 + j*128 + p
    xd = None
    if C_DR:
        xd = x2[:, :, N_BF:].astype(F8)    # [t, m, N_DR]
        xd = xd.reshape(TT, 128, 2 * C_DR, 128).transpose(3, 0, 2, 1)
        xd = np.ascontiguousarray(xd)

    maps = []
    for core in range(N_CORES):
        qwc = qw[core * NS:(core + 1) * NS]            # [NS, IN]
        wb = qwc[:, :N_BF].reshape(NS, C_BF, 128).transpose(2, 1, 0)
        wb = np.ascontiguousarray(wb.astype(BF16))     # [p, c, n]
        m = {"xbf": xb, "wbf": wb}
        if C_DR:
            wd = qwc[:, N_BF:].reshape(NS, 2 * C_DR, 128).transpose(2, 1, 0)
            m["xdr"] = xd
            m["wdr"] = np.ascontiguousarray(wd.astype(F8))
        maps.append(m)
    return maps


def _postprocess(outs: list, bias: np.ndarray) -> np.ndarray:
    out = np.concatenate([np.asarray(o) for o in outs], axis=1)  # [TOK, OUT]
    out = out.reshape(B, S, OUT)
    if np.any(bias):
        out = out + bias.astype(np.float32)
    return out


def _ensure_ntff_hook_shim():
    """concourse's trace path imports antenv.axon_hooks, which is missing in
    this image. Provide the same ctypes-based hook (see trn_agent_boot) so a
    globally-set BASS_TRACE can't crash the run."""
    import sys

    try:
        import antenv.axon_hooks  # noqa: F401
        return
    except ImportError:
        pass

    import contextlib
    import ctypes
    import types

    def _make_hook():
        try:
            lib = ctypes.CDLL("/opt/axon/libaxon_pjrt.so")
        except OSError:
            return None
        if not hasattr(lib, "axon_start_nrt_profile"):
            return None
        lib.axon_start_nrt_profile.argtypes = [
            ctypes.POINTER(ctypes.c_int64), ctypes.c_size_t,
        ]
        lib.axon_start_nrt_profile.restype = ctypes.c_int64
        lib.axon_stop_nrt_profile.argtypes = [ctypes.c_char_p]
        lib.axon_stop_nrt_profile.restype = ctypes.c_int64

        @contextlib.contextmanager
        def _hook(output_dir, device_ids):
            import jax

            jax.devices()
            if device_ids:
                ids = (ctypes.c_int64 * len(device_ids))(*device_ids)
                rc = lib.axon_start_nrt_profile(ids, len(device_ids))
            else:
                rc = lib.axon_start_nrt_profile(None, 0)
            if rc != 0:
                raise RuntimeError(f"axon_start_nrt_profile rc={rc}")
            try:
                yield
            finally:
                lib.axon_stop_nrt_profile(str(output_dir).encode())

        return _hook

    hook = _make_hook()
    mod = types.ModuleType("antenv.axon_hooks")
    mod.get_axon_ntff_profile_hook = lambda: hook
    mod.set_axon_ntff_profile_hook = lambda h: None
    sys.modules["antenv.axon_hooks"] = mod
    try:
        import antenv

        antenv.axon_hooks = mod
    except ImportError:
        pass


def kernel(x: np.ndarray, weight: np.ndarray, bias: np.ndarray) -> np.ndarray:
    from concourse.bass_utils import run_bass_kernel_spmd

    x = np.asarray(x, dtype=np.float32)
    weight = np.asarray(weight, dtype=np.float32)
    bias = np.asarray(bias, dtype=np.float32)

    _ensure_ntff_hook_shim()
    in_maps = _prepare_in_maps(x, weight)
    nc = _get_nc()
    try:
        res = run_bass_kernel_spmd(nc, in_maps, core_ids=list(range(N_CORES)))
    except Exception:
        # transient NRT execute failures have been observed to clear on retry
        import time as _time

        _time.sleep(5)
        res = run_bass_kernel_spmd(nc, in_maps, core_ids=list(range(N_CORES)))
    return _postprocess([r["out"] for r in res.results], bias)


# revision 16
# speedup vs baseline: 1.0899x; 1.0899x over previous
"""BitLinear-1.58 forward on 8 trn2 NeuronCores.

out = x @ qw.T + bias, qw = clip(round(w / (eps + mean|w|)), -1, 1).

Strategy (column-parallel, 1024 out-cols per core):
  - Ternary weights are exact in both bf16 and fp8e4 (values -1/0/1), so the
    only quantization error is on x. The 2048-wide contraction is split:
      * C_BF bf16 chunks (128 cols each): x in bf16, standard matmul,
        1 col/PE-cycle.
      * C_DR fp8 DoubleRow chunks (256 cols each): x in e4m3, 2 cols/PE-cycle
        (perf_mode=DoubleRow packs 2 fp8 rows per PE cell).
    Both chunk types accumulate into the same PSUM group. With C_DR=4 /
    C_BF=8 the kernel does 12 x 512-cycle-ish matmul passes per output tile
    instead of 16, a ~1.28x PE-time reduction, at rel_err ~1.7e-2 (vs the
    2e-2 gate; fp8 columns contribute ~2.65e-2 * sqrt(1024/2048)).
  - Per core: weights stay SBUF-resident (3MB); x streams through SBUF in
    64 token-tiles of 128 tokens; output [128, 1024] f32 written per tile.
"""

import numpy as np
import ml_dtypes

B, S, IN, OUT = 4, 2048, 2048, 8192
N_CORES = 8
TOK = B * S
NS = OUT // N_CORES          # out cols per core
SCALE_EPS = 1e-05

C_DR = 5                     # fp8 DoubleRow chunks (256 in-cols each)
C_BF = 16 - 2 * C_DR         # bf16 chunks (128 in-cols each)
N_DR = C_DR * 256            # fp8-covered in-cols (tail of the 2048)
N_BF = IN - N_DR             # bf16-covered in-cols (head)
TT = TOK // 128              # 64 token tiles
NH = NS // 512               # psum halves (2)

F8 = ml_dtypes.float8_e4m3   # TRN fp8e4 (max 240) == ml_dtypes.float8_e4m3
BF16 = ml_dtypes.bfloat16

_CACHED_NC = None


def _build_nc():
    import concourse.mybir as mybir
    import concourse.tile as tile
    from concourse import bacc

    DR = mybir.MatmulPerfMode.DoubleRow
    nc = bacc.Bacc(None, target_bir_lowering=False)

    xbf = nc.dram_tensor("xbf", [128, TT, C_BF, 128], mybir.dt.bfloat16,
                         kind="ExternalInput")
    wbf = nc.dram_tensor("wbf", [128, C_BF, NS], mybir.dt.bfloat16,
                         kind="ExternalInput")
    if C_DR:
        xdr = nc.dram_tensor("xdr", [128, TT, 2 * C_DR, 128],
                             mybir.dt.float8e4, kind="ExternalInput")
        wdr = nc.dram_tensor("wdr", [128, 2 * C_DR, NS], mybir.dt.float8e4,
                             kind="ExternalInput")
    out = nc.dram_tensor("out", [TOK, NS], mybir.dt.float32,
                         kind="ExternalOutput")

    with tile.TileContext(nc) as tc:
        with (
            tc.tile_pool(name="wres", bufs=1) as wres,
            tc.tile_pool(name="xb_pool", bufs=3) as xb_pool,
            tc.tile_pool(name="xd_pool", bufs=3) as xd_pool,
            tc.tile_pool(name="ostage", bufs=3) as ostage,
            tc.tile_pool(name="psum", bufs=3, space="PSUM") as psum,
            tc.tile_pool(name="warm", bufs=1) as warm_pool,
            tc.tile_pool(name="warm_psum", bufs=1, space="PSUM") as warm_psum,
        ):
            # resident weights; chunk-granular DMAs so compute can start as
            # soon as the first chunk lands
            wdr_t = None
            if C_DR:
                wdr_t = wres.tile([128, 2 * C_DR, NS], mybir.dt.float8e4)
            wbf_t = wres.tile([128, C_BF, NS], mybir.dt.bfloat16)
            # interleave chunk DMAs in consumption order so tile 0 never
            # waits on a weight chunk
            for c in range(max(C_DR, C_BF)):
                if c < C_DR:
                    nc.sync.dma_start(wdr_t[:, 2 * c:2 * c + 2, :],
                                      wdr[:, 2 * c:2 * c + 2, :])
                if c < C_BF:
                    nc.sync.dma_start(wbf_t[:, c:c + 1, :], wbf[:, c:c + 1, :])

            # PE warm-up: dummy matmuls with no data deps keep the PE busy
            # while the first tiles DMA in, releasing the HAM clock gate
            # (2.4 GHz) before the real matmul stream starts.
            wl = warm_pool.tile([128, 512], mybir.dt.bfloat16)
            wp = warm_psum.tile([128, 512], mybir.dt.float32)
            nc.vector.memset(wl[:], 0.0)
            n_warm = 8
            for i in range(n_warm):
                nc.tensor.matmul(
                    wp[:], wl[:, :128], wl[:], start=(i == 0),
                    stop=(i == n_warm - 1)
                )

            for t in range(TT):
                if C_DR:
                    xd = xd_pool.tile([128, 2 * C_DR, 128], mybir.dt.float8e4)
                    nc.scalar.dma_start(xd[:], xdr[:, t, :, :])
                xb = xb_pool.tile([128, C_BF, 128], mybir.dt.bfloat16)
                nc.scalar.dma_start(xb[:], xbf[:, t, :, :])
                ps = [psum.tile([128, 512], mybir.dt.float32, name=f"ps{h}")
                      for h in range(NH)]

                n_chunks = C_DR + C_BF
                ci = 0
                for c in range(C_DR):
                    lhsT = xd[:, 2 * c:2 * c + 2, :]
                    for h in range(NH):
                        nc.tensor.matmul(
                            ps[h][:],
                            lhsT,
                            wdr_t[:, 2 * c:2 * c + 2, h * 512:(h + 1) * 512],
                            start=(ci == 0), stop=(ci == n_chunks - 1),
                            perf_mode=DR,
                        )
                    ci += 1
                for c in range(C_BF):
                    lhsT = xb[:, c:c + 1, :]
                    for h in range(NH):
                        nc.tensor.matmul(
                            ps[h][:],
                            lhsT,
                            wbf_t[:, c:c + 1, h * 512:(h + 1) * 512],
                            start=(ci == 0), stop=(ci == n_chunks - 1),
                        )
                    ci += 1

                stage = ostage.tile([128, NS], mybir.dt.float32)
                for h in range(NH):
                    nc.any.tensor_copy(stage[:, h * 512:(h + 1) * 512],
                                       ps[h][:])
                nc.scalar.dma_start(out[t * 128:(t + 1) * 128, :], stage[:])

    nc.compile()
    return nc


def _get_nc():
    global _CACHED_NC
    if _CACHED_NC is None:
        _CACHED_NC = _build_nc()
    return _CACHED_NC


def _quantize_weight(weight: np.ndarray) -> np.ndarray:
    """Ternarize exactly as the reference does (same jax ops, same backend)."""
    import jax.numpy as jnp

    w = jnp.asarray(weight)
    scale = SCALE_EPS + jnp.mean(jnp.abs(w))
    quant = jnp.clip(jnp.round(w / scale), -1.0, 1.0)
    return np.asarray(quant, dtype=np.float32)


def _prepare_in_maps(x: np.ndarray, weight: np.ndarray):
    qw = _quantize_weight(weight)          # [OUT, IN] ternary fp32

    x2 = x.reshape(TT, 128, IN)            # [t, m, in]

    # bf16 chunks: in-cols [0, N_BF); layout [p, t, c, m]
    xb = x2[:, :, :N_BF].astype(BF16)      # [t, m, N_BF]
    xb = xb.reshape(TT, 128, C_BF, 128).transpose(3, 0, 2, 1)
    xb = np.ascontiguousarray(xb)

    # fp8 chunks: in-cols [N_BF, IN); col = N_BF + c*256 + j*128 + p
    xd = None
    if C_DR:
        xd = x2[:, :, N_BF:].astype(F8)    # [t, m, N_DR]
        xd = xd.reshape(TT, 128, 2 * C_DR, 128).transpose(3, 0, 2, 1)
        xd = np.ascontiguousarray(xd)

    maps = []
    for core in range(N_CORES):
        qwc = qw[core * NS:(core + 1) * NS]            # [NS, IN]
        wb = qwc[:, :N_BF].reshape(NS, C_BF, 128).transpose(2, 1, 0)
        wb = np.ascontiguousarray(wb.astype(BF16))     # [p, c, n]
        m = {"xbf": xb, "wbf": wb}
        if C_DR:
            wd = qwc[:, N_BF:].reshape(NS, 2 * C_DR, 128).transpose(2, 1, 0)
            m["xdr"] = xd
            m["wdr"] = np.ascontiguousarray(wd.astype(F8))
        maps.append(m)
    return maps


def _postprocess(outs: list, bias: np.ndarray) -> np.ndarray:
    out = np.concatenate([np.asarray(o) for o in outs], axis=1)  # [TOK, OUT]
    out = out.reshape(B, S, OUT)
    if np.any(bias):
        out = out + bias.astype(np.float32)
    return out


def _ensure_ntff_hook_shim():
    """concourse's trace path imports antenv.axon_hooks, which is missing in
    this image. Provide the same ctypes-based hook (see trn_agent_boot) so a
    globally-set BASS_TRACE can't crash the run."""
    import sys

    try:
        import antenv.axon_hooks  # noqa: F401
        return
    except ImportError:
        pass

    import contextlib
    import ctypes
    import types

    def _make_hook():
        try:
            lib = ctypes.CDLL("/opt/axon/libaxon_pjrt.so")
        except OSError:
            return None
        if not hasattr(lib, "axon_start_nrt_profile"):
            return None
        lib.axon_start_nrt_profile.argtypes = [
            ctypes.POINTER(ctypes.c_int64), ctypes.c_size_t,
        ]
        lib.axon_start_nrt_profile.restype = ctypes.c_int64
        lib.axon_stop_nrt_profile.argtypes = [ctypes.c_char_p]
        lib.axon_stop_nrt_profile.restype = ctypes.c_int64

        @contextlib.contextmanager
        def _hook(output_dir, device_ids):
            import jax

            jax.devices()
            if device_ids:
                ids = (ctypes.c_int64 * len(device_ids))(*device_ids)
                rc = lib.axon_start_nrt_profile(ids, len(device_ids))
            else:
                rc = lib.axon_start_nrt_profile(None, 0)
            if rc != 0:
                raise RuntimeError(f"axon_start_nrt_profile rc={rc}")
            try:
                yield
            finally:
                lib.axon_stop_nrt_profile(str(output_dir).encode())

        return _hook

    hook = _make_hook()
    mod = types.ModuleType("antenv.axon_hooks")
    mod.get_axon_ntff_profile_hook = lambda: hook
    mod.set_axon_ntff_profile_hook = lambda h: None
    sys.modules["antenv.axon_hooks"] = mod
    try:
        import antenv

        antenv.axon_hooks = mod
    except ImportError:
        pass


def kernel(x: np.ndarray, weight: np.ndarray, bias: np.ndarray) -> np.ndarray:
    from concourse.bass_utils import run_bass_kernel_spmd

    x = np.asarray(x, dtype=np.float32)
    weight = np.asarray(weight, dtype=np.float32)
    bias = np.asarray(bias, dtype=np.float32)

    _ensure_ntff_hook_shim()
    in_maps = _prepare_in_maps(x, weight)
    nc = _get_nc()
    try:
        res = run_bass_kernel_spmd(nc, in_maps, core_ids=list(range(N_CORES)))
    except Exception:
        # transient NRT execute failures have been observed to clear on retry
        import time as _time

        _time.sleep(5)
        res = run_bass_kernel_spmd(nc, in_maps, core_ids=list(range(N_CORES)))
    return _postprocess([r["out"] for r in res.results], bias)
